# revision 17
# baseline (speedup 1.0000x reference)
"""EnergyTransformer Bass kernel for 8 trn2 NeuronCores (v2, pipelined).

Sharding: core c -> batch b=c//2, token-half t=c%2 (256 tokens each),
head-half h0=t*8.  Token universe per batch is kept in GLOBAL (agb)
order [r0t0, r0t1, r1t0, r1t1] on BOTH pair members, which makes every
address rank-agnostic (SPMD-safe).

Per step:
  phase A (attention, all 512 tokens, 8 local heads):
    gT_all assembled from the two per-tile AllGathers issued during the
    PREVIOUS step's phase B.  Q/K projections emitted in nt order
    {0,2,1,3} so the {0,2} columns (which depend only on AG#0) start
    while AG#1 may still be in flight.  Scores/exp/t1/t2 as v1.
    Back-projection emitted in chunk order {0,2} then {1,3}; each chunk
    feeds a ReduceScatter (chunk c = global tiles {c, 2+c}), so RS#0
    overlaps the second half of back-projection and RS#1 overlaps the
    first hopfield tile.
  phase B (hopfield, token-local, tile-outer):
    for each local 128-token tile: x += rs_c; LN; hT = relu(alpha *
    xiT @ g2T) per 128-mem slice; dg += hT.T @ xi (xi streamed); x +=
    dg; then the NEXT step's LN + gT transpose + AllGather for that
    tile is issued immediately, hiding AG#0 under the other tile's
    hopfield.

LN rstd uses ACT Rsqrt (one table set) and hopfield relu runs on ACT
(relu is in every table set), so the activation table flips only twice
per step instead of eight times.
"""

import numpy as np

import concourse.bass as bass
import concourse.bacc as bacc
import concourse.mybir as mybir
import concourse.tile as tile
from concourse.bass_utils import run_bass_kernel_spmd
from concourse.masks import make_identity

F32 = mybir.dt.float32
BF16 = mybir.dt.bfloat16
AF = mybir.ActivationFunctionType
ALU = mybir.AluOpType
DEFAULT_MDT = "bf16"

B, N, D, H, DH, M = 4, 512, 1024, 16, 64, 4096
STEPS = 12
ALPHA = 0.125
EPS = 1e-5
SCALE = 1.0 / np.sqrt(DH)  # 0.125

NLOC = N // 2          # tokens per core = 256
TT = NLOC // 128       # token tiles per core = 2
NT = N // 128          # token tiles per batch = 4
DT = D // 128          # d tiles = 8
HL = H // 2            # heads per core = 8
HP = HL // 2           # head pairs per core = 4
MS = M // 128          # memory slices = 32
PAIRS = [[0, 1], [2, 3], [4, 5], [6, 7]]

_CACHE = {}


def _pbcast(ap, parts):
    """Prepend a stride-0 partition dim of size `parts` to an AP."""
    return bass.AP(tensor=ap.tensor, offset=ap.offset,
                   ap=[[0, parts]] + [list(d) for d in ap.ap])


def _layer_norm(nc, lnp, x_ap, g_ap, eps_t, gamma_bc, beta_bc):
    """g = gamma*(x-mean)*rsqrt(var+eps)+beta for one [128, D] tile."""
    st = lnp.tile([128, 2, 6], F32, tag="ln_stats")
    mv = lnp.tile([128, 2], F32, tag="ln_mv")
    rst = lnp.tile([128, 1], F32, tag="ln_rstd")
    for sg in range(2):
        nc.vector.bn_stats(out=st[:, sg, :], in_=x_ap[:, sg * 512:(sg + 1) * 512])
    nc.vector.bn_aggr(out=mv, in_=st)
    nc.scalar.activation(out=rst, in_=mv[:, 1:2], func=AF.Sqrt, bias=eps_t[:])
    nc.vector.reciprocal(out=rst, in_=rst)
    nc.vector.tensor_scalar(out=g_ap, in0=x_ap, scalar1=mv[:, 0:1], scalar2=rst,
                            op0=ALU.subtract, op1=ALU.mult)
    if gamma_bc is not None:
        nc.vector.tensor_mul(out=g_ap, in0=g_ap, in1=gamma_bc[:])
    if beta_bc is not None:
        nc.vector.tensor_add(out=g_ap, in0=g_ap, in1=beta_bc[:])


def build_program(apply_gamma=False, apply_beta=False, steps=STEPS, mdt=DEFAULT_MDT,
                  no_comm=False, resident_xi=True, dg_bf16=True, hop_fp8=True):
    MDT = BF16 if mdt == "bf16" else F32
    FP8 = mybir.dt.float8e4
    HDT = FP8 if hop_fp8 else MDT  # hopfield operand dtype

    def mm(ap):
        return ap.bitcast(mybir.dt.float32r) if mdt == "f32r" else ap

    nc = bacc.Bacc("TRN2", num_devices=8, debug=False, target_bir_lowering=False)

    # ---- I/O ----
    x_in = nc.dram_tensor("x_loc", [NLOC, D], F32, kind="ExternalInput")
    wq_p = nc.dram_tensor("wq_proj", [D, HP * 128], MDT, kind="ExternalInput")
    wk_p = nc.dram_tensor("wk_proj", [D, HP * 128], MDT, kind="ExternalInput")
    wqt = nc.dram_tensor("wqT_bp", [HP * 128, D], MDT, kind="ExternalInput")
    wkt = nc.dram_tensor("wkT_bp", [HP * 128, D], MDT, kind="ExternalInput")
    xiT_d = nc.dram_tensor("xiT", [D, M], HDT, kind="ExternalInput")
    xi_d = nc.dram_tensor("xi", [M, D], HDT, kind="ExternalInput")
    gamma_d = nc.dram_tensor("gamma", [D], F32, kind="ExternalInput")
    beta_d = nc.dram_tensor("beta", [D], F32, kind="ExternalInput")
    ss_d = nc.dram_tensor("skip_scale", [1], F32, kind="ExternalInput")
    out_d = nc.dram_tensor("out", [NLOC, D], F32, kind="ExternalOutput")

    WDT = BF16 if dg_bf16 else F32

    with tile.TileContext(nc) as tc:
        import contextlib
        ctx = contextlib.ExitStack()
        with ctx:
            consts = ctx.enter_context(tc.tile_pool(name="consts", bufs=1))
            wpool = ctx.enter_context(tc.tile_pool(name="weights", bufs=1))
            xpool = ctx.enter_context(tc.tile_pool(name="xstate", bufs=1))
            gpool = ctx.enter_context(tc.tile_pool(name="g", bufs=2))
            gtl = ctx.enter_context(tc.tile_pool(name="gtl", bufs=2))
            gta = ctx.enter_context(tc.tile_pool(name="gta", bufs=2))
            g2p = ctx.enter_context(tc.tile_pool(name="g2t", bufs=2))
            qkt = ctx.enter_context(tc.tile_pool(name="qkt", bufs=3))
            ppool = ctx.enter_context(
                tc.tile_pool(name="pexp", bufs=1 if hop_fp8 else 2))
            spool = ctx.enter_context(tc.tile_pool(name="small", bufs=2))
            lnp = ctx.enter_context(tc.tile_pool(name="ln", bufs=2))
            t12 = ctx.enter_context(tc.tile_pool(name="t12", bufs=1))
            rbcp = ctx.enter_context(tc.tile_pool(name="rbc", bufs=2))
            hpool = ctx.enter_context(tc.tile_pool(name="hT", bufs=4))
            strm = ctx.enter_context(tc.tile_pool(name="strm", bufs=3))
            dgsb = ctx.enter_context(tc.tile_pool(name="dgsb", bufs=2))
            dram = ctx.enter_context(tc.tile_pool(name="dram", bufs=2, space="DRAM"))
            ps_mm = ctx.enter_context(tc.tile_pool(name="ps_mm", bufs=2, space="PSUM"))
            ps_aux = ctx.enter_context(tc.tile_pool(name="ps_aux", bufs=2, space="PSUM"))
            ps_dg = ctx.enter_context(tc.tile_pool(name="ps_dg", bufs=2, space="PSUM"))

            # ---- constants ----
            ident = consts.tile([128, 128], F32)
            make_identity(nc, ident[:])
            if MDT is F32:
                ident_m = ident
            else:
                ident_m = consts.tile([128, 128], MDT)
                make_identity(nc, ident_m[:])
            eps_t = consts.tile([128, 1], F32)
            nc.vector.memset(eps_t[:], EPS)
            ss_bc = consts.tile([128, 1], F32)
            nc.gpsimd.dma_start(out=ss_bc[:], in_=ss_d[:].to_broadcast((128, 1)))
            gamma_bc = beta_bc = None
            if apply_gamma:
                gamma_bc = consts.tile([128, D], F32)
                nc.gpsimd.dma_start(out=gamma_bc[:],
                                    in_=gamma_d[:].to_broadcast((128, D)))
            if apply_beta:
                beta_bc = consts.tile([128, D], F32)
                nc.gpsimd.dma_start(out=beta_bc[:],
                                    in_=beta_d[:].to_broadcast((128, D)))

            # ---- weights resident in SBUF ----
            wq_sb = wpool.tile([128, DT, HP * 128], MDT)
            wk_sb = wpool.tile([128, DT, HP * 128], MDT)
            nc.sync.dma_start(out=wq_sb[:], in_=wq_p[:].rearrange("(dt p) c -> p dt c", p=128))
            nc.sync.dma_start(out=wk_sb[:], in_=wk_p[:].rearrange("(dt p) c -> p dt c", p=128))
            wqt_sb = wpool.tile([128, HP, D], MDT)
            wkt_sb = wpool.tile([128, HP, D], MDT)
            nc.sync.dma_start(out=wqt_sb[:], in_=wqt[:].rearrange("(hp p) d -> p hp d", p=128))
            nc.sync.dma_start(out=wkt_sb[:], in_=wkt[:].rearrange("(hp p) d -> p hp d", p=128))

            xiT_res = wpool.tile([128, DT, M], HDT)
            nc.sync.dma_start(
                out=xiT_res[:],
                in_=xiT_d[:].rearrange("(dt p) m -> p dt m", p=128))
            xi_res = None
            if hop_fp8:
                xi_res = wpool.tile([128, MS, D], HDT)
                nc.sync.dma_start(
                    out=xi_res[:],
                    in_=xi_d[:].rearrange("(ms p) d -> p ms d", p=128))

            # ---- x state ----
            x_tiles = []
            for tt in range(TT):
                xt = xpool.tile([128, D], F32, tag=f"x{tt}")
                nc.sync.dma_start(out=xt[:], in_=x_in[tt * 128:(tt + 1) * 128, :])
                x_tiles.append(xt)

            def ln_gt_ag(tt):
                """LN(x_tiles[tt]) -> transpose -> gtb[tt] -> AG#tt.
                Returns the agb tile the AG writes."""
                g = gpool.tile([128, D], F32, tag="g")
                _layer_norm(nc, lnp, x_tiles[tt][:], g[:], eps_t,
                            gamma_bc, beta_bc)
                gT = gtl.tile([128, DT, 128], MDT, tag=f"gtl{tt}")
                for dt in range(DT):
                    tp = ps_aux.tile([128, 128], F32, tag="aux")
                    nc.tensor.transpose(tp[:], g[:, dt * 128:(dt + 1) * 128],
                                        ident[:])
                    nc.vector.tensor_copy(out=gT[:, dt, :], in_=tp[:])
                gtb = dram.tile([D, 128], MDT, tag=f"gtb{tt}")
                agb = dram.tile([2 * D, 128], MDT, tag=f"agb{tt}")
                nc.sync.dma_start(
                    out=gtb[:].rearrange("(dt p) c -> p dt c", p=128), in_=gT[:])
                if no_comm:
                    nc.gpsimd.dma_start(out=agb[0:D, :], in_=gtb[:])
                    nc.gpsimd.dma_start(out=agb[D:2 * D, :], in_=gtb[:])
                else:
                    nc.gpsimd.collective_compute(
                        "AllGather", ALU.bypass, replica_groups=PAIRS,
                        ins=[gtb[:]], outs=[agb[:]])
                return agb

            # prologue: initial LN + AG for both tiles
            agbs = [ln_gt_ag(tt) for tt in range(TT)]

            for step in range(steps):
                # ============ phase A : attention ============
                # assemble gT_all (global order) from the two AGs
                # global tile g: agb#(g%2) rows [(g//2)*D : (g//2+1)*D]
                gT_all = gta.tile([128, DT, N], MDT, tag="gtall")
                for g_t in (0, 2, 1, 3):
                    src = agbs[g_t % 2]
                    r = g_t // 2
                    nc.sync.dma_start(
                        out=gT_all[:, :, g_t * 128:(g_t + 1) * 128],
                        in_=src[r * D:(r + 1) * D, :].rearrange(
                            "(dt p) c -> p dt c", p=128))

                # per-step attention buffers
                den = spool.tile([128, HL * 4], F32, tag="den")
                recip = spool.tile([128, HL * 4], F32, tag="recip")
                t1T = t12.tile([128, HP, N], MDT, tag="t1T")
                t2T = t12.tile([128, HP, N], MDT, tag="t2T")

                # chunk-0 back-projection accumulates inside the hp loop so
                # RS#0 can issue as soon as the last hp's scores are done
                pdg_c0 = []
                for nt in (0, 2):
                    pdg_nt = ps_dg.tile([128, D], F32, tag="dg")
                    pdg_c0.append(pdg_nt)

                for hp in range(HP):
                    # QT/KT projections for head pair (rows 0-63 even head,
                    # 64-127 odd head); nt order {0,2,1,3} so the AG#0-
                    # dependent columns can start while AG#1 is in flight.
                    qt = qkt.tile([128, N], MDT, tag="qt")
                    kt = qkt.tile([128, N], MDT, tag="kt")
                    for (dst, wsb) in ((qt, wq_sb), (kt, wk_sb)):
                        pmm = ps_mm.tile([128, N], F32, tag="mm")
                        for nt in (0, 2, 1, 3):
                            cs = slice(nt * 128, (nt + 1) * 128)
                            for dt in range(DT):
                                nc.tensor.matmul(
                                    pmm[:, cs], mm(wsb[:, dt, hp * 128:(hp + 1) * 128]),
                                    mm(gT_all[:, dt, cs]),
                                    start=(dt == 0), stop=(dt == DT - 1))
                        nc.scalar.copy(out=dst[:], in_=pmm[:])

                    rbc = rbcp.tile([128, N], F32, tag="rbc")
                    scr = dram.tile([2, N], F32, tag="scr")
                    pt, pu, qn, qu, ku = [], [], [], [], []
                    for hw in range(2):
                        pt_h = ppool.tile([128, NT, N], MDT, tag=f"pt{hw}")
                        pu_h = ppool.tile([128, NT, N], MDT, tag=f"pu{hw}")
                        qn_h = spool.tile([128, NT, DH], MDT, tag=f"qn{hw}")
                        qu_h = spool.tile([128, NT, DH], MDT, tag=f"qu{hw}")
                        ku_h = spool.tile([128, NT, DH], MDT, tag=f"ku{hw}")
                        pt.append(pt_h); pu.append(pu_h)
                        qn.append(qn_h); qu.append(qu_h); ku.append(ku_h)
                    # Q/K transposed copies via DMA-transpose (scalar HWDGE):
                    # qu/ku[p, it, e] = q/k[i=it*128+p, hb+e]
                    for hw in range(2):
                        hb = hw * 64
                        nc.sync.dma_start(out=ku[hw][:], in_=kt[hb:hb + 64, :],
                                          transpose=True)
                        nc.sync.dma_start(out=qu[hw][:], in_=qt[hb:hb + 64, :],
                                          transpose=True)
                    # aT scores (queries on partitions): exp + denominator;
                    # pu = exp(a)^T obtained by DMA-transposing pt tiles
                    for jt in range(NT):
                        for hw in range(2):
                            hb = hw * 64
                            c4 = (hp * 2 + hw) * 4
                            pa = ps_mm.tile([128, N], F32, tag="mm")
                            nc.tensor.matmul(
                                pa[:], mm(qt[hb:hb + 64, jt * 128:(jt + 1) * 128]),
                                mm(kt[hb:hb + 64, :]), start=True, stop=True)
                            nc.scalar.activation(
                                out=pt[hw][:, jt, :], in_=pa[:], func=AF.Exp,
                                scale=float(SCALE),
                                accum_out=den[:, c4 + jt:c4 + jt + 1])
                            nc.sync.dma_start(
                                out=pu[hw][:, :, jt * 128:(jt + 1) * 128],
                                in_=pt[hw][:, jt, :], transpose=True)
                    for hw in range(2):
                        c4 = (hp * 2 + hw) * 4
                        nc.vector.reciprocal(out=recip[:, c4:c4 + 4],
                                             in_=den[:, c4:c4 + 4])
                        nc.sync.dma_start(
                            out=scr[hw, :].rearrange("(jt p) -> p jt", p=128),
                            in_=recip[:, c4:c4 + 4])
                        nc.sync.dma_start(
                            out=rbc[hw * 64:hw * 64 + 64, :],
                            in_=_pbcast(scr[hw, :], 64))
                    # normalized Q (per-query 1/den)
                    for jt in range(NT):
                        for hw in range(2):
                            c4 = (hp * 2 + hw) * 4
                            nc.vector.tensor_scalar_mul(
                                out=qn[hw][:, jt, :], in0=qu[hw][:, jt, :],
                                scalar1=recip[:, c4 + jt:c4 + jt + 1])
                    # raw t2T (needs per-query recip via rbc)
                    r1 = ps_aux.tile([128, N], F32, tag="aux")
                    for it in range(NT):
                        for hw in range(2):
                            hb = hw * 64
                            nc.tensor.matmul(
                                r1[hb:hb + 64, :], mm(ku[hw][:, it, :]),
                                mm(pu[hw][:, it, :]),
                                start=(it == 0), stop=(it == NT - 1),
                                tile_position=(0, hb) if hb else None)
                    for hw in range(2):
                        hb = hw * 64
                        nc.vector.tensor_mul(out=t1T[hb:hb + 64, hp, :],
                                             in0=r1[hb:hb + 64, :],
                                             in1=rbc[hb:hb + 64, :])
                    # t1T path (qn already normalized)
                    r2 = ps_aux.tile([128, N], F32, tag="aux")
                    for jt in range(NT):
                        for hw in range(2):
                            hb = hw * 64
                            nc.tensor.matmul(
                                r2[hb:hb + 64, :], mm(qn[hw][:, jt, :]),
                                mm(pt[hw][:, jt, :]),
                                start=(jt == 0), stop=(jt == NT - 1),
                                tile_position=(0, hb) if hb else None)
                    for hw in range(2):
                        hb = hw * 64
                        nc.vector.tensor_copy(out=t2T[hb:hb + 64, hp, :],
                                              in_=r2[hb:hb + 64, :])

                    # chunk-0 back-projection partials for this hp
                    for half, nt in enumerate((0, 2)):
                        for nh in range(2):
                            for k, (tsb, wsb) in enumerate(
                                    ((t1T, wqt_sb), (t2T, wkt_sb))):
                                nc.tensor.matmul(
                                    pdg_c0[half][:, nh * 512:(nh + 1) * 512],
                                    mm(tsb[:, hp, nt * 128:(nt + 1) * 128]),
                                    mm(wsb[:, hp, nh * 512:(nh + 1) * 512]),
                                    start=(hp == 0 and k == 0),
                                    stop=(hp == HP - 1 and k == 1))

                # chunk 0: drain + RS#0 (chunk c = global tiles {c, 2+c};
                # row 0-127 = tile c, row 128-255 = tile 2+c)
                rsbs = []
                dgb0 = dram.tile([2 * 128, D], WDT, tag="dgb0")
                rsb0 = dram.tile([128, D], WDT, tag="rsb0")
                for half in range(2):
                    dsb = dgsb.tile([128, D], WDT, tag="dgsb")
                    nc.scalar.copy(out=dsb[:], in_=pdg_c0[half][:])
                    nc.sync.dma_start(
                        out=dgb0[half * 128:(half + 1) * 128, :], in_=dsb[:])
                if no_comm:
                    nc.gpsimd.dma_start(out=rsb0[:], in_=dgb0[0:128, :])
                else:
                    nc.gpsimd.collective_compute(
                        "ReduceScatter", ALU.add, replica_groups=PAIRS,
                        ins=[dgb0[:]], outs=[rsb0[:]])
                rsbs.append(rsb0)

                # chunk 1: back-projection + RS#1
                dgb1 = dram.tile([2 * 128, D], WDT, tag="dgb1")
                rsb1 = dram.tile([128, D], WDT, tag="rsb1")
                for half, nt in enumerate((1, 3)):
                    pdg = ps_dg.tile([128, D], F32, tag="dg")
                    for nh in range(2):
                        k = 0
                        for hp in range(HP):
                            for (tsb, wsb) in ((t1T, wqt_sb), (t2T, wkt_sb)):
                                nc.tensor.matmul(
                                    pdg[:, nh * 512:(nh + 1) * 512],
                                    mm(tsb[:, hp, nt * 128:(nt + 1) * 128]),
                                    mm(wsb[:, hp, nh * 512:(nh + 1) * 512]),
                                    start=(k == 0), stop=(k == 2 * HP - 1))
                                k += 1
                    dsb = dgsb.tile([128, D], WDT, tag="dgsb")
                    nc.scalar.copy(out=dsb[:], in_=pdg[:])
                    nc.sync.dma_start(
                        out=dgb1[half * 128:(half + 1) * 128, :], in_=dsb[:])
                if no_comm:
                    nc.gpsimd.dma_start(out=rsb1[:], in_=dgb1[0:128, :])
                else:
                    nc.gpsimd.collective_compute(
                        "ReduceScatter", ALU.add, replica_groups=PAIRS,
                        ins=[dgb1[:]], outs=[rsb1[:]])
                rsbs.append(rsb1)

                # ============ phase B : hopfield, tile-outer ============
                last = step == steps - 1
                new_agbs = []
                # both x accumulations issued up front so the Pool queue
                # never blocks them behind a collective
                for tt in range(TT):
                    nc.gpsimd.dma_start(out=x_tiles[tt][:], in_=rsbs[tt][:],
                                        accum_op=ALU.add)
                for tt in range(TT):
                    g2 = gpool.tile([128, D], F32, tag="g")
                    _layer_norm(nc, lnp, x_tiles[tt][:], g2[:], eps_t,
                                gamma_bc, beta_bc)
                    g2T = g2p.tile([128, DT, 128], HDT, tag="g2t")
                    for dt in range(DT):
                        tp = ps_aux.tile([128, 128], F32, tag="aux")
                        nc.tensor.transpose(tp[:], g2[:, dt * 128:(dt + 1) * 128],
                                            ident[:])
                        nc.vector.tensor_copy(out=g2T[:, dt, :], in_=tp[:])

                    pdgh = ps_dg.tile([128, D], F32, tag="dg")
                    for ms in range(MS):
                        ph = ps_aux.tile([128, 128], F32, tag="aux")
                        for dt in range(DT):
                            nc.tensor.matmul(
                                ph[:], mm(xiT_res[:, dt, ms * 128:(ms + 1) * 128]),
                                mm(g2T[:, dt, :]),
                                start=(dt == 0), stop=(dt == DT - 1))
                        hT = hpool.tile([128, 128], HDT, tag="hT")
                        nc.scalar.activation(out=hT[:], in_=ph[:], func=AF.Relu,
                                             scale=ALPHA)
                        if hop_fp8:
                            xi_t = xi_res[:, ms, :]
                        else:
                            xi_tile = strm.tile([128, D], MDT, tag="xi")
                            nc.sync.dma_start(
                                out=xi_tile[:],
                                in_=xi_d[ms * 128:(ms + 1) * 128, :])
                            xi_t = xi_tile[:]
                        for nh in range(2):
                            nc.tensor.matmul(
                                pdgh[:, nh * 512:(nh + 1) * 512],
                                mm(hT[:]), mm(xi_t[:, nh * 512:(nh + 1) * 512]),
                                start=(ms == 0), stop=(ms == MS - 1))
                    nc.vector.tensor_add(out=x_tiles[tt][:], in0=x_tiles[tt][:],
                                         in1=pdgh[:])
                    if not last:
                        new_agbs.append(ln_gt_ag(tt))
                if not last:
                    agbs = new_agbs

            # ---- final skip connection ----
            for tt in range(TT):
                res = gpool.tile([128, D], F32, tag="g")
                nc.sync.dma_start(out=res[:], in_=x_in[tt * 128:(tt + 1) * 128, :])
                nc.scalar.activation(out=res[:], in_=res[:], func=AF.Copy,
                                     scale=ss_bc[:])
                nc.vector.tensor_add(out=res[:], in0=res[:], in1=x_tiles[tt][:])
                nc.sync.dma_start(out=out_d[tt * 128:(tt + 1) * 128, :], in_=res[:])

    nc.compile()
    return nc


def _prep_inputs(x, gamma, beta, wq, wk, xi, skip_scale, mdt=DEFAULT_MDT,
                 hop_fp8=True):
    """Build per-core input maps (host-side sharding + weight packing)."""
    import ml_dtypes
    if mdt == "bf16":
        wdt = ml_dtypes.bfloat16
    else:
        wdt = np.float32
    hdt = ml_dtypes.float8_e4m3 if hop_fp8 else wdt
    x = np.asarray(x, np.float32)
    wq = np.asarray(wq, np.float32)
    wk = np.asarray(wk, np.float32)
    xi_f = np.asarray(xi, np.float32)
    xiT = np.ascontiguousarray(xi_f.T).astype(hdt)
    xi = np.ascontiguousarray(xi_f).astype(hdt)
    in_maps = []
    for c in range(8):
        b, t = c // 2, c % 2
        h0 = t * HL
        wq_loc = wq[h0:h0 + HL]          # [8, 1024, 64]
        wk_loc = wk[h0:h0 + HL]
        # projection packing: [1024, hp*128] with 2 heads side by side
        wq_proj = np.concatenate([wq_loc[i] for i in range(HL)], axis=1)
        wk_proj = np.concatenate([wk_loc[i] for i in range(HL)], axis=1)
        # back-projection: [hp*128, 1024], rows = 2 heads' (alpha*W^T) stacked
        wqT_bp = np.concatenate([ALPHA * wq_loc[i].T for i in range(HL)], axis=0)
        wkT_bp = np.concatenate([ALPHA * wk_loc[i].T for i in range(HL)], axis=0)
        in_maps.append({
            "x_loc": np.ascontiguousarray(x[b, t * NLOC:(t + 1) * NLOC]),
            "wq_proj": np.ascontiguousarray(wq_proj).astype(wdt),
            "wk_proj": np.ascontiguousarray(wk_proj).astype(wdt),
            "wqT_bp": np.ascontiguousarray(wqT_bp).astype(wdt),
            "wkT_bp": np.ascontiguousarray(wkT_bp).astype(wdt),
            "xiT": xiT,
            "xi": xi,
            "gamma": np.asarray(gamma, np.float32),
            "beta": np.asarray(beta, np.float32),
            "skip_scale": np.asarray(skip_scale, np.float32).reshape(1),
        })
    return in_maps


def run(inputs, trace=False, mdt=DEFAULT_MDT, **bkw):
    gamma = np.asarray(inputs["gamma"], np.float32)
    beta = np.asarray(inputs["beta"], np.float32)
    apply_gamma = not np.all(gamma == 1.0)
    apply_beta = not np.all(beta == 0.0)
    key = (apply_gamma, apply_beta, mdt, tuple(sorted(bkw.items())))
    if key not in _CACHE:
        _CACHE[key] = build_program(apply_gamma, apply_beta, mdt=mdt, **bkw)
    nc = _CACHE[key]
    in_maps = _prep_inputs(**inputs, mdt=mdt,
                           hop_fp8=bkw.get("hop_fp8", True))
    res = run_bass_kernel_spmd(nc, in_maps, list(range(8)), trace=trace)
    out = np.empty((B, N, D), np.float32)
    for c in range(8):
        b, t = c // 2, c % 2
        out[b, t * NLOC:(t + 1) * NLOC] = res.results[c]["out"]
    return out, res


def kernel(**inputs) -> np.ndarray:
    out, _ = run(inputs, mdt="bf16")
    return out


if __name__ == "__main__":
    pass
